# revision 6
# baseline (speedup 1.0000x reference)
"""Trainium2 Bass kernel for causal top-K cosine-similarity GNN message passing.

Module: delta = gelu(mix*x + (1-mix)*msg) * scale, where msg is the mean of
the K=8 causally-preceding neighbors with highest cosine similarity.

Strategy (8 NeuronCores, SPMD):
  - batch b -> core pair (2b, 2b+1). Per batch, 16 query units of 256 rows;
    even core takes units {15,13,...,1}, odd {14,12,...,0}. All cores run an
    identical program under the shared key-width schedule WS=[16,14,...,2]
    (x256 keys); causal masking is data-driven (additive -1e30 mask input),
    so padding columns self-mask.
  - sim tiles via PE matmul (fp32r operands: full speed, ~2.8e-6 rms noise);
    per-row 8th-largest via the DVE max8 instruction; adjacency built as
    A = (sim >= max(tau, -2)) * msgc in one DVE op (msgc = (1-mix)/K folded
    into the compare), selecting exactly the top-8 (ties measure-zero).
  - msg matmul: A tiles PE-transposed to put keys on partitions, then
    fp8e4 DoubleRow matmuls (256-key contraction per pass, 2x PE rate);
    x quantized to e4m3 on host. Epilogue fused:
    pm = mix*xq + pm (DVE scalar_tensor_tensor), gelu (ACT), *scale (DVE).
  - All DRAM inputs host-packed so every SBUF tile loads with one
    dma_start of contiguous >=2KB lines (fast start, low issue overhead).
  - Rows 0..6 of each batch (fewer than 8 neighbors) are fixed up exactly on
    the host (28 of 16384 rows).
"""

import os
import sys

if "/opt/trn_rl_repo" not in sys.path:
    sys.path.insert(0, "/opt/trn_rl_repo")

_NOGELU = bool(os.environ.get("KERNEL_SIM_NOGELU"))  # CoreSim lacks Gelu

import numpy as np
import ml_dtypes

import concourse.bacc as bacc
import concourse.mybir as mybir
import concourse.tile as tile
from concourse.bass_utils import run_bass_kernel_spmd
from concourse.masks import make_identity

B, T, D, K = 4, 4096, 1024, 8
U = 256                      # unit (query block) size
WS = [16, 14, 12, 10, 8, 6, 4, 2]   # per-slot key width, in units
NSLOT = len(WS)
QPC = NSLOT * U              # query rows per core (2048)
NRES = 15                    # resident key units (unit 15 only ever = even q, via xqnt)
NEG = -1.0e30

f32 = mybir.dt.float32
f32r = mybir.dt.float32r
bf16 = mybir.dt.bfloat16
fp8 = mybir.dt.float8e4
AF = mybir.ActivationFunctionType
ALU = mybir.AluOpType
PM = mybir.MatmulPerfMode

SIM_DT = "f32r"              # "f32r" (fast, tf32-like) or "f32" (exact, 4x slower sim)
MSG_DT = os.environ.get("KERNEL_MSG_DT", "fp8")   # "fp8" (DoubleRow) or "bf16"

_PROG_CACHE = {}


def _build_program(sim_dt_key, msg_dt_key, trivial_affine, mix, scale):
    sdt = f32r if sim_dt_key == "f32r" else f32
    use_fp8 = msg_dt_key == "fp8"
    mdt = fp8 if use_fp8 else bf16
    msgc = (1.0 - mix) / K

    nc = bacc.Bacc("TRN2", target_bir_lowering=False, debug=False)

    # unit-major packed: [p, k*U+c] = xn[u*U+c, k*128+p]
    xnt_d = nc.dram_tensor("xnt", [NRES, 128, 8 * U], sdt, kind="ExternalInput")
    xqnt_d = nc.dram_tensor("xqnt", [NSLOT, 128, 8 * U], sdt, kind="ExternalInput")
    # epilogue queries: [s][p, h*D+d] = x[q(s)*U + h*128 + p, d]
    xq_d = nc.dram_tensor("xq", [NSLOT, 128, 2 * D], bf16, kind="ExternalInput")
    if use_fp8:
        # key groups of 256 rows: [g][p, i*D+d] = x[g*256 + i*128 + p, d]
        xk_d = nc.dram_tensor("xk", [T // 256, 128, 2 * D], fp8, kind="ExternalInput")
    else:
        xk_d = nc.dram_tensor("xk", [T, D], bf16, kind="ExternalInput")
    cm_d = nc.dram_tensor("cmask", [2, 128, 2 * U], f32, kind="ExternalInput")
    if not trivial_affine:
        gain_d = nc.dram_tensor("gain", [1, D], f32, kind="ExternalInput")
        bias_d = nc.dram_tensor("bias", [1, D], f32, kind="ExternalInput")
    out_d = nc.dram_tensor("out", [QPC, D], f32, kind="ExternalOutput")

    with tile.TileContext(nc) as tc:
        with (
            tc.tile_pool(name="res", bufs=1) as res_pool,
            tc.tile_pool(name="simp", bufs=2) as sim_pool,
            tc.tile_pool(name="qw", bufs=2) as qw_pool,
            tc.tile_pool(name="xqep", bufs=2) as xqe_pool,
            tc.tile_pool(name="xkp", bufs=4) as xk_pool,
            tc.tile_pool(name="atp", bufs=2) as at_pool,
            tc.tile_pool(name="atrp", bufs=3) as atr_pool,
            tc.tile_pool(name="m8p", bufs=2) as m8_pool,
            tc.tile_pool(name="otp", bufs=3) as ot_pool,
            tc.tile_pool(name="ps_sim", bufs=2, space="PSUM") as psim_pool,
            tc.tile_pool(name="ps_tr", bufs=2, space="PSUM") as ptr_pool,
            tc.tile_pool(name="ps_msg", bufs=4, space="PSUM") as pmsg_pool,
        ):
            # ---- resident keys: 8 tiles of <=2 units, delivery-ordered so
            # early slots start before the whole 15 MiB lands ----
            XS = [(u0, min(2, NRES - u0)) for u0 in range(0, NRES, 2)]
            xnt_sbs = []
            for ti, (u0, nu) in enumerate(XS):
                t = res_pool.tile([128, nu, 8 * U], sdt, tag=f"xnt{ti}",
                                  name=f"xnt{ti}")
                for uo in range(nu):
                    nc.gpsimd.dma_start(out=t[:, uo:uo + 1, :],
                                        in_=xnt_d[u0 + uo])
                xnt_sbs.append(t)

            def key_rhs(k, u0, ncols):
                # [128, nu, 256] AP over key cols [u0*U, u0*U+ncols) of chunk k
                nu = ncols // U
                ti = u0 // 2
                uo = u0 - 2 * ti
                assert uo + nu <= XS[ti][1], (u0, ncols)
                return xnt_sbs[ti][:, uo:uo + nu, k * U:(k + 1) * U]

            cm_sb = res_pool.tile([128, 2 * 2 * U], f32, tag="cm")
            for h in range(2):
                nc.sync.dma_start(out=cm_sb[:, h * 2 * U:(h + 1) * 2 * U], in_=cm_d[h])
            ident = res_pool.tile([128, 128], bf16, tag="ident")
            make_identity(nc, ident[:])
            if not trivial_affine:
                gb_sb = res_pool.tile([128, 2 * D], f32, tag="gb")
                g1 = res_pool.tile([1, 2 * D], f32, tag="g1")
                nc.sync.dma_start(out=g1[:, 0:D], in_=gain_d[:])
                nc.sync.dma_start(out=g1[:, D:2 * D], in_=bias_d[:])
                nc.vector.partition_broadcast(gb_sb[:], g1[:])

            def emit_qk(s):
                qk = qw_pool.tile([128, 8 * U], sdt, tag="qw", name=f"qk_{s}")
                nc.sync.dma_start(out=qk[:], in_=xqnt_d[s])
                xqe = xqe_pool.tile([128, 2 * D], bf16, tag="xqe", name=f"xqe_{s}")
                nc.sync.dma_start(out=xqe[:], in_=xq_d[s])
                return qk, xqe

            order = list(reversed(range(NSLOT)))
            qk_next = emit_qk(order[0])
            for si, s in enumerate(order):
                W = WS[s]
                NJ = W // 2          # 512-wide sim tiles (last one is split)
                sim_t = [None, None]
                tauc = [None, None]

                qk, xqe = qk_next
                if si + 1 < len(order):
                    qk_next = emit_qk(order[si + 1])

                # ---- phase 1: sim + max8 threshold, per stripe ----
                for h in range(2):
                    sim_t[h] = sim_pool.tile([128, 16 * U], f32, tag="sim", name=f"sim_{s}_{h}")
                    m8all = m8_pool.tile([128, 8 * 8], f32, tag="m8all")
                    for jg in range(NJ):
                        psim = psim_pool.tile([128, 512], f32, tag="psim", name=f"psim_{s}_{h}_{jg}")
                        if jg < NJ - 1:
                            for k in range(8):
                                nc.tensor.matmul(
                                    psim[:], qk[:, k * U + h * 128: k * U + h * 128 + 128],
                                    key_rhs(k, 2 * jg, 512),
                                    start=(k == 0), stop=(k == 7))
                        else:
                            # split final tile: two sequential accumulation
                            # groups in the same bank (interleaving is illegal).
                            # qk-self group first: no xnt dependency, so the
                            # very first slot can start before keys land.
                            for k in range(8):
                                nc.tensor.matmul(
                                    psim[:, U:2 * U], qk[:, k * U + h * 128: k * U + h * 128 + 128],
                                    qk[:, k * U:(k + 1) * U],
                                    start=(k == 0), stop=(k == 7))
                            for k in range(8):
                                nc.tensor.matmul(
                                    psim[:, 0:U], qk[:, k * U + h * 128: k * U + h * 128 + 128],
                                    key_rhs(k, W - 2, U),
                                    start=(k == 0), stop=(k == 7))
                        dst = sim_t[h][:, jg * 512:(jg + 1) * 512]
                        if jg < NJ - 1:
                            nc.scalar.copy(dst, psim[:])
                        else:
                            nc.vector.tensor_add(dst, psim[:], cm_sb[:, h * 2 * U:(h + 1) * 2 * U])
                        nc.vector.max(out=m8all[:, jg * 8:(jg + 1) * 8], in_=dst)
                    m8f = m8_pool.tile([128, 8], f32, tag="m8f")
                    nc.vector.max(out=m8f[:], in_=m8all[:, 0:NJ * 8])
                    tauc[h] = m8_pool.tile([128, 1], f32, tag="tauc", name=f"tauc_{s}_{h}")
                    nc.vector.tensor_scalar_max(tauc[h][:], m8f[:, 7:8], -2.0)

                # ---- phase 2: A-build + transpose + msg matmul ----
                # PE order T(h0,jg) M(h0,jg) T(h1,jg) M(h1,jg): the first
                # msg group covers tau(h1)'s DVE latency, so the PE never
                # waits at the stripe boundary.
                pmsg = [[pmsg_pool.tile([128, 512], f32, tag="pmsg", name=f"pmsg_{s}_{h}_{dh}")
                         for dh in range(2)] for h in range(2)]
                for jg in range(NJ):
                    xkt = []
                    for g in range(2):
                        jj8 = jg * 2 + g
                        if use_fp8:
                            xt = xk_pool.tile([128, 2, D], fp8, tag="xk")
                            nc.sync.dma_start(out=xt[:], in_=xk_d[jj8])
                        else:
                            xt = [None, None]
                            for i in range(2):
                                xt[i] = xk_pool.tile([128, D], bf16, tag="xk")
                                nc.sync.dma_start(
                                    out=xt[i][:],
                                    in_=xk_d[(jj8 * 2 + i) * 128:(jj8 * 2 + i + 1) * 128, :])
                        xkt.append(xt)
                    for h in range(2):
                        a_t = at_pool.tile([128, 512], bf16, tag="at")
                        nc.vector.tensor_scalar(
                            a_t[:], sim_t[h][:, jg * 512:(jg + 1) * 512],
                            tauc[h][:], float(msgc), op0=ALU.is_ge, op1=ALU.mult)
                        if use_fp8:
                            # transpose in bf16 (fp8 PE transpose needs
                            # stride-2 outputs); ACT copy converts to fp8
                            ptr = ptr_pool.tile([128, 4, 128], bf16, tag="ptr")
                            for t in range(4):
                                nc.tensor.transpose(ptr[:, t:t + 1, :],
                                                    a_t[:, t * 128:(t + 1) * 128], ident[:])
                            atr = atr_pool.tile([128, 4, 128], fp8, tag="atr",
                                                name=f"atr_{s}_{jg}_{h}")
                            nc.scalar.copy(atr[:], ptr[:])
                            for g in range(2):
                                for dh in range(2):
                                    nc.tensor.matmul(
                                        pmsg[h][dh][:], atr[:, 2 * g:2 * g + 2, :],
                                        xkt[g][:, :, dh * 512:(dh + 1) * 512],
                                        perf_mode=PM.DoubleRow,
                                        start=(jg == 0 and g == 0),
                                        stop=(jg == NJ - 1 and g == 1))
                        else:
                            ptr = ptr_pool.tile([128, 512], bf16, tag="ptr")
                            for t in range(4):
                                nc.tensor.transpose(ptr[:, t * 128:(t + 1) * 128],
                                                    a_t[:, t * 128:(t + 1) * 128], ident[:])
                            atr = atr_pool.tile([128, 512], bf16, tag="atr",
                                                name=f"atr_{s}_{jg}_{h}")
                            nc.scalar.copy(atr[:], ptr[:])
                            for g in range(2):
                                for i in range(2):
                                    jj = jg * 4 + g * 2 + i
                                    for dh in range(2):
                                        nc.tensor.matmul(
                                            pmsg[h][dh][:],
                                            atr[:, (g * 2 + i) * 128:(g * 2 + i + 1) * 128],
                                            xkt[g][i][:, dh * 512:(dh + 1) * 512],
                                            start=(jj == 0), stop=(jj == 4 * NJ - 1))

                # ---- phase 3: fused epilogue ----
                for h in range(2):
                    for dh in range(2):
                        pm = pmsg[h][dh]
                        nc.vector.scalar_tensor_tensor(
                            pm[:], xqe[:, h * D + dh * 512: h * D + (dh + 1) * 512],
                            float(mix), pm[:], op0=ALU.mult, op1=ALU.add)
                        if not trivial_affine:
                            nc.vector.tensor_mul(pm[:], pm[:], gb_sb[:, dh * 512:(dh + 1) * 512])
                            nc.vector.tensor_add(pm[:], pm[:], gb_sb[:, D + dh * 512: D + (dh + 1) * 512])
                        ot = ot_pool.tile([128, 512], f32, tag="ot")
                        nc.scalar.activation(ot[:], pm[:],
                                             AF.Identity if _NOGELU else AF.Gelu)
                        nc.vector.tensor_scalar_mul(ot[:], ot[:], float(scale))
                        nc.sync.dma_start(
                            out=out_d[s * U + h * 128: s * U + (h + 1) * 128,
                                      dh * 512:(dh + 1) * 512],
                            in_=ot[:])
    nc.compile()
    return nc


def _gelu_exact(z):
    from scipy.special import erf
    z64 = z.astype(np.float64)
    return (0.5 * z64 * (1.0 + erf(z64 / np.sqrt(2.0)))).astype(np.float32)


def _pack_unit(rows):
    # rows: (256, 1024) f32 -> [128, 8*256]: out[p, k*U+c] = rows[c, k*128+p]
    return np.ascontiguousarray(
        rows.T.reshape(8, 128, U).transpose(1, 0, 2).reshape(128, 8 * U))


def kernel(x, gain, bias, log_mix, log_scale):
    x = np.asarray(x, dtype=np.float32)
    gain = np.asarray(gain, dtype=np.float32)
    bias = np.asarray(bias, dtype=np.float32)
    mix = float(1.0 / (1.0 + np.exp(-np.float64(log_mix))))
    scale = float(np.log1p(np.exp(np.float64(log_scale))) + 0.01)
    trivial = bool(np.all(gain == 1.0) and np.all(bias == 0.0))

    key = (SIM_DT, MSG_DT, trivial, round(mix, 12), round(scale, 12))
    if key not in _PROG_CACHE:
        _PROG_CACHE[key] = _build_program(SIM_DT, MSG_DT, trivial, mix, scale)
    nc = _PROG_CACHE[key]
    use_fp8 = MSG_DT == "fp8"

    norms = np.sqrt((x.astype(np.float32) ** 2).sum(-1, keepdims=True)).astype(np.float32)
    xn = x / (norms + np.float32(1e-8))

    in_maps = []
    qunits = []  # per core: list of q unit index per slot
    for c in range(8):
        b, p = c // 2, c % 2
        kus = [WS[s] - 1 - p for s in range(NSLOT)]
        qunits.append(kus)
        xb, xnb = x[b], xn[b]
        xnt_np = np.stack([_pack_unit(xnb[u * U:(u + 1) * U]) for u in range(NRES)])
        xqnt_np = np.stack([_pack_unit(xnb[ku * U:(ku + 1) * U]) for ku in kus])
        xq_np = np.stack([
            np.concatenate([xb[ku * U:ku * U + 128], xb[ku * U + 128:(ku + 1) * U]], axis=1)
            for ku in kus]).astype(ml_dtypes.bfloat16)
        if use_fp8:
            xk_np = np.stack([
                np.concatenate([xb[g * 256:g * 256 + 128], xb[g * 256 + 128:(g + 1) * 256]], axis=1)
                for g in range(T // 256)]).astype(ml_dtypes.float8_e4m3)
        else:
            xk_np = xb.astype(ml_dtypes.bfloat16)
        r = np.arange(128)[:, None]
        f = np.arange(2 * U)[None, :]
        cm = np.zeros((2, 128, 2 * U), dtype=np.float32)
        for h in range(2):
            row = h * 128 + r
            allowed = f <= (row + U) if p == 0 else f <= row
            cm[h] = np.where(allowed, 0.0, NEG)
        m = {"xnt": xnt_np, "xqnt": xqnt_np, "xk": xk_np, "xq": xq_np, "cmask": cm}
        if not trivial:
            m["gain"] = gain.reshape(1, D)
            m["bias"] = bias.reshape(1, D)
        in_maps.append(m)

    global _LAST_IN_MAPS
    _LAST_IN_MAPS = in_maps
    res = run_bass_kernel_spmd(nc, in_maps, list(range(8)), trace=False)

    y = np.empty((B, T, D), dtype=np.float32)
    for c in range(8):
        b = c // 2
        oc = res.results[c]["out"]
        for s, ku in enumerate(qunits[c]):
            y[b, ku * U:(ku + 1) * U] = oc[s * U:(s + 1) * U]

    # exact host fixup for rows with fewer than K neighbors (q < 7)
    for b in range(B):
        nq = K - 1
        msg = np.cumsum(x[b, :nq], axis=0) / np.arange(1, nq + 1, dtype=np.float32)[:, None]
        blended = np.float32(mix) * x[b, :nq] + np.float32(1.0 - mix) * msg
        y[b, :nq] = _gelu_exact(blended * gain + bias) * np.float32(scale)

    return y


# revision 13
# speedup vs baseline: 1.0091x; 1.0091x over previous
"""Trainium2 Bass kernel for causal top-K cosine-similarity GNN message passing.

Module: delta = gelu(mix*x + (1-mix)*msg) * scale, where msg is the mean of
the K=8 causally-preceding neighbors with highest cosine similarity.

Strategy (8 NeuronCores, SPMD):
  - batch b -> core pair (2b, 2b+1). Per batch, 16 query units of 256 rows;
    even core takes units {15,13,...,1}, odd {14,12,...,0}. All cores run an
    identical program under the shared key-width schedule WS=[16,14,...,2]
    (x256 keys); causal masking is data-driven (additive -1e30 mask input),
    so padding columns self-mask.
  - sim tiles via PE matmul (fp32r operands: full speed, ~2.8e-6 rms noise);
    per-row 8th-largest via the DVE max8 instruction; adjacency built as
    A = (sim >= max(tau, -2)) * msgc in one DVE op (msgc = (1-mix)/K folded
    into the compare), selecting exactly the top-8 (ties measure-zero).
  - msg matmul: A tiles PE-transposed to put keys on partitions, then
    fp8e4 DoubleRow matmuls (256-key contraction per pass, 2x PE rate);
    x quantized to e4m3 on host. Epilogue fused:
    pm = mix*xq + pm (DVE scalar_tensor_tensor), gelu (ACT), *scale (DVE).
  - All DRAM inputs host-packed so every SBUF tile loads with one
    dma_start of contiguous >=2KB lines (fast start, low issue overhead).
  - Rows 0..6 of each batch (fewer than 8 neighbors) are fixed up exactly on
    the host (28 of 16384 rows).
"""

import os
import sys

if "/opt/trn_rl_repo" not in sys.path:
    sys.path.insert(0, "/opt/trn_rl_repo")

_NOGELU = bool(os.environ.get("KERNEL_SIM_NOGELU"))  # CoreSim lacks Gelu

import numpy as np
import ml_dtypes

import concourse.bacc as bacc
import concourse.mybir as mybir
import concourse.tile as tile
from concourse.bass_utils import run_bass_kernel_spmd
from concourse.masks import make_identity

B, T, D, K = 4, 4096, 1024, 8
U = 256                      # unit (query block) size
WS = [16, 14, 12, 10, 8, 6, 4, 2]   # per-slot key width, in units
NSLOT = len(WS)
QPC = NSLOT * U              # query rows per core (2048)
NRES = 15                    # resident key units (unit 15 only ever = even q, via xqnt)
NEG = -1.0e30

f32 = mybir.dt.float32
f32r = mybir.dt.float32r
bf16 = mybir.dt.bfloat16
fp16 = mybir.dt.float16
fp8 = mybir.dt.float8e4
AF = mybir.ActivationFunctionType
ALU = mybir.AluOpType
PM = mybir.MatmulPerfMode

SIM_DT = "f32r"              # "f32r" (fast, tf32-like) or "f32" (exact, 4x slower sim)
MSG_DT = os.environ.get("KERNEL_MSG_DT", "fp8")   # "fp8" (DoubleRow) or "bf16"

_PROG_CACHE = {}


def _build_program(sim_dt_key, msg_dt_key, trivial_affine, mix, scale):
    sdt = f32r if sim_dt_key == "f32r" else f32
    use_fp8 = msg_dt_key == "fp8"
    mdt = fp8 if use_fp8 else bf16
    msgc = (1.0 - mix) / K

    nc = bacc.Bacc("TRN2", target_bir_lowering=False, debug=False)

    # unit-major packed: [p, k*U+c] = xn[u*U+c, k*128+p]
    xnt_d = nc.dram_tensor("xnt", [NRES, 128, 8 * U], sdt, kind="ExternalInput")
    xqnt_d = nc.dram_tensor("xqnt", [NSLOT, 128, 8 * U], sdt, kind="ExternalInput")
    # epilogue queries: [s][p, h*D+d] = x[q(s)*U + h*128 + p, d]
    xq_d = nc.dram_tensor("xq", [NSLOT, 128, 2 * D], bf16, kind="ExternalInput")
    if use_fp8:
        # key groups of 256 rows: [g][p, i*D+d] = x[g*256 + i*128 + p, d]
        xk_d = nc.dram_tensor("xk", [T // 256, 128, 2 * D], fp8, kind="ExternalInput")
    else:
        xk_d = nc.dram_tensor("xk", [T, D], bf16, kind="ExternalInput")
    cm_d = nc.dram_tensor("cmask", [2, 128, 2 * U], f32, kind="ExternalInput")
    if not trivial_affine:
        gain_d = nc.dram_tensor("gain", [1, D], f32, kind="ExternalInput")
        bias_d = nc.dram_tensor("bias", [1, D], f32, kind="ExternalInput")
    out_d = nc.dram_tensor("out", [QPC, D], bf16, kind="ExternalOutput")

    with tile.TileContext(nc) as tc:
        with (
            tc.tile_pool(name="res", bufs=1) as res_pool,
            tc.tile_pool(name="simp", bufs=2) as sim_pool,
            tc.tile_pool(name="qw", bufs=2) as qw_pool,
            tc.tile_pool(name="xqep", bufs=2) as xqe_pool,
            tc.tile_pool(name="xkp", bufs=4) as xk_pool,
            tc.tile_pool(name="atp", bufs=2) as at_pool,
            tc.tile_pool(name="atrp", bufs=3) as atr_pool,
            tc.tile_pool(name="m8p", bufs=2) as m8_pool,
            tc.tile_pool(name="otp", bufs=3) as ot_pool,
            tc.tile_pool(name="ps_sim", bufs=2, space="PSUM") as psim_pool,
            tc.tile_pool(name="ps_tr", bufs=2, space="PSUM") as ptr_pool,
            tc.tile_pool(name="ps_msg", bufs=4, space="PSUM") as pmsg_pool,
        ):
            # ---- resident keys: 8 tiles of <=2 units. Units 0-5 issue up
            # front; the rest just-in-time (2 per slot) so the 15 MiB bulk
            # does not starve qk/xk on the saturated DMA engines ----
            XS = [(u0, min(2, NRES - u0)) for u0 in range(0, NRES, 2)]
            xnt_sbs = [res_pool.tile([128, nu, 8 * U], sdt, tag=f"xnt{ti}",
                                     name=f"xnt{ti}")
                       for ti, (u0, nu) in enumerate(XS)]
            _units_issued = [0]

            def issue_xnt(upto):
                while _units_issued[0] < min(upto, NRES):
                    u = _units_issued[0]
                    nc.gpsimd.dma_start(out=xnt_sbs[u // 2][:, u % 2:u % 2 + 1, :],
                                        in_=xnt_d[u])
                    _units_issued[0] = u + 1

            def key_rhs(k, u0, ncols):
                # [128, nu, 256] AP over key cols [u0*U, u0*U+ncols) of chunk k
                nu = ncols // U
                ti = u0 // 2
                uo = u0 - 2 * ti
                assert uo + nu <= XS[ti][1], (u0, ncols)
                return xnt_sbs[ti][:, uo:uo + nu, k * U:(k + 1) * U]

            def emit_qk(s):
                qk = qw_pool.tile([128, 8 * U], sdt, tag="qw", name=f"qk_{s}")
                nc.sync.dma_start(out=qk[:], in_=xqnt_d[s])
                xqe = xqe_pool.tile([128, 2 * D], bf16, tag="xqe", name=f"xqe_{s}")
                nc.sync.dma_start(out=xqe[:], in_=xq_d[s])
                return qk, xqe

            order = list(reversed(range(NSLOT)))
            # first slot's queries lead the sync queue: the PE start gates
            # on them (qk-self sim group needs no resident keys)
            qk_next = emit_qk(order[0])
            cm_sb = res_pool.tile([128, 2 * 2 * U], f32, tag="cm")
            for h in range(2):
                nc.sync.dma_start(out=cm_sb[:, h * 2 * U:(h + 1) * 2 * U], in_=cm_d[h])
            ident = res_pool.tile([128, 128], bf16, tag="ident")
            make_identity(nc, ident[:])
            issue_xnt(6)
            if not trivial_affine:
                gb_sb = res_pool.tile([128, 2 * D], f32, tag="gb")
                g1 = res_pool.tile([1, 2 * D], f32, tag="g1")
                nc.sync.dma_start(out=g1[:, 0:D], in_=gain_d[:])
                nc.sync.dma_start(out=g1[:, D:2 * D], in_=bias_d[:])
                nc.vector.partition_broadcast(gb_sb[:], g1[:])

            for si, s in enumerate(order):
                W = WS[s]
                NJ = W // 2          # 512-wide sim tiles
                # split final tile only where needed: W=16 (unit 15 is not
                # resident) and W=2 (qk-self start needs no resident keys)
                split_last = W in (2, 16)
                sim_t = [None, None]
                tauc = [None, None]

                qk, xqe = qk_next
                if si + 1 < len(order):
                    qk_next = emit_qk(order[si + 1])
                issue_xnt(W + 4)

                # ---- phase 1: sim + max8 threshold, per stripe ----
                for h in range(2):
                    sim_t[h] = sim_pool.tile([128, 16 * U], f32, tag="sim", name=f"sim_{s}_{h}")
                    m8all = m8_pool.tile([128, 8 * 8], f32, tag="m8all")
                    for jg in range(NJ):
                        psim = psim_pool.tile([128, 512], f32, tag="psim", name=f"psim_{s}_{h}_{jg}")
                        if jg < NJ - 1 or not split_last:
                            for k in range(8):
                                nc.tensor.matmul(
                                    psim[:], qk[:, k * U + h * 128: k * U + h * 128 + 128],
                                    key_rhs(k, 2 * jg, 512),
                                    start=(k == 0), stop=(k == 7))
                        else:
                            # split final tile: two sequential accumulation
                            # groups in the same bank (interleaving is illegal).
                            # qk-self group first: no xnt dependency, so the
                            # very first slot can start before keys land.
                            for k in range(8):
                                nc.tensor.matmul(
                                    psim[:, U:2 * U], qk[:, k * U + h * 128: k * U + h * 128 + 128],
                                    qk[:, k * U:(k + 1) * U],
                                    start=(k == 0), stop=(k == 7))
                            for k in range(8):
                                nc.tensor.matmul(
                                    psim[:, 0:U], qk[:, k * U + h * 128: k * U + h * 128 + 128],
                                    key_rhs(k, W - 2, U),
                                    start=(k == 0), stop=(k == 7))
                        dst = sim_t[h][:, jg * 512:(jg + 1) * 512]
                        if jg < NJ - 1:
                            nc.scalar.copy(dst, psim[:])
                        else:
                            nc.vector.tensor_add(dst, psim[:], cm_sb[:, h * 2 * U:(h + 1) * 2 * U])
                        nc.vector.max(out=m8all[:, jg * 8:(jg + 1) * 8], in_=dst)
                    m8f = m8_pool.tile([128, 8], f32, tag="m8f")
                    nc.vector.max(out=m8f[:], in_=m8all[:, 0:NJ * 8])
                    tauc[h] = m8_pool.tile([128, 1], f32, tag="tauc", name=f"tauc_{s}_{h}")
                    nc.vector.tensor_scalar_max(tauc[h][:], m8f[:, 7:8], -2.0)

                # ---- phase 2: A-build + transpose + msg matmul ----
                # PE order T(h0,jg) M(h0,jg) T(h1,jg) M(h1,jg): the first
                # msg group covers tau(h1)'s DVE latency, so the PE never
                # waits at the stripe boundary.
                pmsg = [[pmsg_pool.tile([128, 512], f32, tag="pmsg", name=f"pmsg_{s}_{h}_{dh}")
                         for dh in range(2)] for h in range(2)]
                for jg in range(NJ):
                    xkt = []
                    for g in range(2):
                        jj8 = jg * 2 + g
                        if use_fp8:
                            xt = xk_pool.tile([128, 2, D], fp8, tag="xk")
                            nc.sync.dma_start(out=xt[:], in_=xk_d[jj8])
                        else:
                            xt = [None, None]
                            for i in range(2):
                                xt[i] = xk_pool.tile([128, D], bf16, tag="xk")
                                nc.sync.dma_start(
                                    out=xt[i][:],
                                    in_=xk_d[(jj8 * 2 + i) * 128:(jj8 * 2 + i + 1) * 128, :])
                        xkt.append(xt)
                    for h in range(2):
                        a_t = at_pool.tile([128, 512], bf16, tag="at")
                        nc.vector.tensor_scalar(
                            a_t[:], sim_t[h][:, jg * 512:(jg + 1) * 512],
                            tauc[h][:], float(msgc), op0=ALU.is_ge, op1=ALU.mult)
                        if use_fp8:
                            # transpose in bf16 (fp8 PE transpose needs
                            # stride-2 outputs); ACT copy converts to fp8
                            ptr = ptr_pool.tile([128, 4, 128], bf16, tag="ptr")
                            for t in range(4):
                                nc.tensor.transpose(ptr[:, t:t + 1, :],
                                                    a_t[:, t * 128:(t + 1) * 128], ident[:])
                            atr = atr_pool.tile([128, 4, 128], fp8, tag="atr",
                                                name=f"atr_{s}_{jg}_{h}")
                            nc.scalar.copy(atr[:], ptr[:])
                            for g in range(2):
                                for dh in range(2):
                                    nc.tensor.matmul(
                                        pmsg[h][dh][:], atr[:, 2 * g:2 * g + 2, :],
                                        xkt[g][:, :, dh * 512:(dh + 1) * 512],
                                        perf_mode=PM.DoubleRow,
                                        start=(jg == 0 and g == 0),
                                        stop=(jg == NJ - 1 and g == 1))
                        else:
                            ptr = ptr_pool.tile([128, 512], bf16, tag="ptr")
                            for t in range(4):
                                nc.tensor.transpose(ptr[:, t * 128:(t + 1) * 128],
                                                    a_t[:, t * 128:(t + 1) * 128], ident[:])
                            atr = atr_pool.tile([128, 512], bf16, tag="atr",
                                                name=f"atr_{s}_{jg}_{h}")
                            nc.scalar.copy(atr[:], ptr[:])
                            for g in range(2):
                                for i in range(2):
                                    jj = jg * 4 + g * 2 + i
                                    for dh in range(2):
                                        nc.tensor.matmul(
                                            pmsg[h][dh][:],
                                            atr[:, (g * 2 + i) * 128:(g * 2 + i + 1) * 128],
                                            xkt[g][i][:, dh * 512:(dh + 1) * 512],
                                            start=(jj == 0), stop=(jj == 4 * NJ - 1))

                # ---- phase 3: fused epilogue ----
                for h in range(2):
                    for dh in range(2):
                        pm = pmsg[h][dh]
                        nc.vector.scalar_tensor_tensor(
                            pm[:], xqe[:, h * D + dh * 512: h * D + (dh + 1) * 512],
                            float(mix), pm[:], op0=ALU.mult, op1=ALU.add)
                        if not trivial_affine:
                            nc.vector.tensor_mul(pm[:], pm[:], gb_sb[:, dh * 512:(dh + 1) * 512])
                            nc.vector.tensor_add(pm[:], pm[:], gb_sb[:, D + dh * 512: D + (dh + 1) * 512])
                        ot = ot_pool.tile([128, 512], bf16, tag="ot")
                        nc.scalar.activation(ot[:], pm[:],
                                             AF.Identity if _NOGELU else AF.Gelu)
                        nc.vector.tensor_scalar_mul(ot[:], ot[:], float(scale))
                        nc.sync.dma_start(
                            out=out_d[s * U + h * 128: s * U + (h + 1) * 128,
                                      dh * 512:(dh + 1) * 512],
                            in_=ot[:])
    nc.compile()
    return nc


def _gelu_exact(z):
    from scipy.special import erf
    z64 = z.astype(np.float64)
    return (0.5 * z64 * (1.0 + erf(z64 / np.sqrt(2.0)))).astype(np.float32)


def _pack_unit(rows):
    # rows: (256, 1024) f32 -> [128, 8*256]: out[p, k*U+c] = rows[c, k*128+p]
    return np.ascontiguousarray(
        rows.T.reshape(8, 128, U).transpose(1, 0, 2).reshape(128, 8 * U))


def kernel(x, gain, bias, log_mix, log_scale):
    x = np.asarray(x, dtype=np.float32)
    gain = np.asarray(gain, dtype=np.float32)
    bias = np.asarray(bias, dtype=np.float32)
    mix = float(1.0 / (1.0 + np.exp(-np.float64(log_mix))))
    scale = float(np.log1p(np.exp(np.float64(log_scale))) + 0.01)
    trivial = bool(np.all(gain == 1.0) and np.all(bias == 0.0))

    key = (SIM_DT, MSG_DT, trivial, round(mix, 12), round(scale, 12))
    if key not in _PROG_CACHE:
        _PROG_CACHE[key] = _build_program(SIM_DT, MSG_DT, trivial, mix, scale)
    nc = _PROG_CACHE[key]
    use_fp8 = MSG_DT == "fp8"

    norms = np.sqrt((x.astype(np.float32) ** 2).sum(-1, keepdims=True)).astype(np.float32)
    xn = x / (norms + np.float32(1e-8))

    in_maps = []
    qunits = []  # per core: list of q unit index per slot
    for c in range(8):
        b, p = c // 2, c % 2
        kus = [WS[s] - 1 - p for s in range(NSLOT)]
        qunits.append(kus)
        xb, xnb = x[b], xn[b]
        xnt_np = np.stack([_pack_unit(xnb[u * U:(u + 1) * U]) for u in range(NRES)])
        xqnt_np = np.stack([_pack_unit(xnb[ku * U:(ku + 1) * U]) for ku in kus])
        xq_np = np.stack([
            np.concatenate([xb[ku * U:ku * U + 128], xb[ku * U + 128:(ku + 1) * U]], axis=1)
            for ku in kus]).astype(ml_dtypes.bfloat16)
        if use_fp8:
            xk_np = np.stack([
                np.concatenate([xb[g * 256:g * 256 + 128], xb[g * 256 + 128:(g + 1) * 256]], axis=1)
                for g in range(T // 256)]).astype(ml_dtypes.float8_e4m3)
        else:
            xk_np = xb.astype(ml_dtypes.bfloat16)
        r = np.arange(128)[:, None]
        f = np.arange(2 * U)[None, :]
        cm = np.zeros((2, 128, 2 * U), dtype=np.float32)
        for h in range(2):
            row = h * 128 + r
            allowed = f <= (row + U) if p == 0 else f <= row
            cm[h] = np.where(allowed, 0.0, NEG)
        m = {"xnt": xnt_np, "xqnt": xqnt_np, "xk": xk_np, "xq": xq_np, "cmask": cm}
        if not trivial:
            m["gain"] = gain.reshape(1, D)
            m["bias"] = bias.reshape(1, D)
        in_maps.append(m)

    global _LAST_IN_MAPS
    _LAST_IN_MAPS = in_maps
    res = run_bass_kernel_spmd(nc, in_maps, list(range(8)), trace=False)

    y = np.empty((B, T, D), dtype=np.float32)
    for c in range(8):
        b = c // 2
        oc = np.asarray(res.results[c]["out"]).astype(np.float32)
        for s, ku in enumerate(qunits[c]):
            y[b, ku * U:(ku + 1) * U] = oc[s * U:(s + 1) * U]

    # exact host fixup for rows with fewer than K neighbors (q < 7)
    for b in range(B):
        nq = K - 1
        msg = np.cumsum(x[b, :nq], axis=0) / np.arange(1, nq + 1, dtype=np.float32)[:, None]
        blended = np.float32(mix) * x[b, :nq] + np.float32(1.0 - mix) * msg
        y[b, :nq] = _gelu_exact(blended * gain + bias) * np.float32(scale)

    return y


# revision 16
# speedup vs baseline: 1.1189x; 1.1089x over previous
"""Trainium2 Bass kernel for causal top-K cosine-similarity GNN message passing.

Module: delta = gelu(mix*x + (1-mix)*msg) * scale, where msg is the mean of
the K=8 causally-preceding neighbors with highest cosine similarity.

Strategy (8 NeuronCores, SPMD):
  - batch b -> core pair (2b, 2b+1). Per batch, 16 query units of 256 rows;
    even core takes units {15,13,...,1}, odd {14,12,...,0}. All cores run an
    identical program under the shared key-width schedule WS=[16,14,...,2]
    (x256 keys); causal masking is data-driven (additive -1e30 mask input),
    so padding columns self-mask.
  - sim tiles via PE matmul (fp32r operands: full speed, ~2.8e-6 rms noise);
    per-row 8th-largest via the DVE max8 instruction; adjacency built as
    A = (sim >= max(tau, -2)) * msgc in one DVE op (msgc = (1-mix)/K folded
    into the compare), selecting exactly the top-8 (ties measure-zero).
  - msg matmul: A tiles PE-transposed to put keys on partitions, then
    fp8e4 DoubleRow matmuls (256-key contraction per pass, 2x PE rate);
    x quantized to e4m3 on host. Epilogue fused:
    pm = mix*xq + pm (DVE scalar_tensor_tensor), gelu (ACT), *scale (DVE).
  - All DRAM inputs host-packed so every SBUF tile loads with one
    dma_start of contiguous >=2KB lines (fast start, low issue overhead).
  - Rows 0..6 of each batch (fewer than 8 neighbors) are fixed up exactly on
    the host (28 of 16384 rows).
"""

import os
import sys

if "/opt/trn_rl_repo" not in sys.path:
    sys.path.insert(0, "/opt/trn_rl_repo")

_NOGELU = bool(os.environ.get("KERNEL_SIM_NOGELU"))  # CoreSim lacks Gelu

import numpy as np
import ml_dtypes

import concourse.bacc as bacc
import concourse.mybir as mybir
import concourse.tile as tile
from concourse.bass_utils import run_bass_kernel_spmd
from concourse.masks import make_identity

B, T, D, K = 4, 4096, 1024, 8
U = 256                      # unit (query block) size
WS = [16, 14, 12, 10, 8, 6, 4, 2]   # per-slot key width, in units
NSLOT = len(WS)
QPC = NSLOT * U              # query rows per core (2048)
NRES = 15                    # resident key units (unit 15 only ever = even q, via xqnt)
NEG = -1.0e30

f32 = mybir.dt.float32
f32r = mybir.dt.float32r
bf16 = mybir.dt.bfloat16
fp16 = mybir.dt.float16
fp8 = mybir.dt.float8e4
AF = mybir.ActivationFunctionType
ALU = mybir.AluOpType
PM = mybir.MatmulPerfMode

SIM_DT = os.environ.get("KERNEL_SIM_DT", "fp16")  # "fp16" (half DMA) | "f32r" (tf32-like) | "f32"
MSG_DT = os.environ.get("KERNEL_MSG_DT", "fp8")   # "fp8" (DoubleRow) or "bf16"

_PROG_CACHE = {}


def _build_program(sim_dt_key, msg_dt_key, trivial_affine, mix, scale):
    sdt = {"f32r": f32r, "f32": f32, "fp16": fp16}[sim_dt_key]
    use_fp8 = msg_dt_key == "fp8"
    mdt = fp8 if use_fp8 else bf16
    msgc = (1.0 - mix) / K

    nc = bacc.Bacc("TRN2", target_bir_lowering=False, debug=False)

    # unit-major packed: [p, k*U+c] = xn[u*U+c, k*128+p]
    xnt_d = nc.dram_tensor("xnt", [NRES, 128, 8 * U], sdt, kind="ExternalInput")
    xqnt_d = nc.dram_tensor("xqnt", [NSLOT, 128, 8 * U], sdt, kind="ExternalInput")
    # epilogue queries: [s][p, h*D+d] = x[q(s)*U + h*128 + p, d]
    xq_d = nc.dram_tensor("xq", [NSLOT, 128, 2 * D], bf16, kind="ExternalInput")
    if use_fp8:
        # key groups of 256 rows: [g][p, i*D+d] = x[g*256 + i*128 + p, d]
        xk_d = nc.dram_tensor("xk", [T // 256, 128, 2 * D], fp8, kind="ExternalInput")
    else:
        xk_d = nc.dram_tensor("xk", [T, D], bf16, kind="ExternalInput")
    cm_d = nc.dram_tensor("cmask", [2, 128, 2 * U], f32, kind="ExternalInput")
    if not trivial_affine:
        gain_d = nc.dram_tensor("gain", [1, D], f32, kind="ExternalInput")
        bias_d = nc.dram_tensor("bias", [1, D], f32, kind="ExternalInput")
    out_d = nc.dram_tensor("out", [QPC, D], bf16, kind="ExternalOutput")

    with tile.TileContext(nc) as tc:
        with (
            tc.tile_pool(name="res", bufs=1) as res_pool,
            tc.tile_pool(name="simp", bufs=2) as sim_pool,
            tc.tile_pool(name="qw", bufs=2) as qw_pool,
            tc.tile_pool(name="xqep", bufs=2) as xqe_pool,
            tc.tile_pool(name="xkp", bufs=4) as xk_pool,
            tc.tile_pool(name="atp", bufs=2) as at_pool,
            tc.tile_pool(name="atrp", bufs=3) as atr_pool,
            tc.tile_pool(name="m8p", bufs=2) as m8_pool,
            tc.tile_pool(name="otp", bufs=3) as ot_pool,
            tc.tile_pool(name="ps_sim", bufs=2, space="PSUM") as psim_pool,
            tc.tile_pool(name="ps_tr", bufs=2, space="PSUM") as ptr_pool,
            tc.tile_pool(name="ps_msg", bufs=4, space="PSUM") as pmsg_pool,
        ):
            # ---- resident keys: 8 tiles of <=2 units. Units 0-5 issue up
            # front; the rest just-in-time (2 per slot) so the 15 MiB bulk
            # does not starve qk/xk on the saturated DMA engines ----
            XS = [(u0, min(2, NRES - u0)) for u0 in range(0, NRES, 2)]
            xnt_sbs = [res_pool.tile([128, nu, 8 * U], sdt, tag=f"xnt{ti}",
                                     name=f"xnt{ti}")
                       for ti, (u0, nu) in enumerate(XS)]
            _units_issued = [0]

            def issue_xnt(upto):
                while _units_issued[0] < min(upto, NRES):
                    u = _units_issued[0]
                    nc.gpsimd.dma_start(out=xnt_sbs[u // 2][:, u % 2:u % 2 + 1, :],
                                        in_=xnt_d[u])
                    _units_issued[0] = u + 1

            def key_rhs(k, u0, ncols):
                # [128, nu, 256] AP over key cols [u0*U, u0*U+ncols) of chunk k
                nu = ncols // U
                ti = u0 // 2
                uo = u0 - 2 * ti
                assert uo + nu <= XS[ti][1], (u0, ncols)
                return xnt_sbs[ti][:, uo:uo + nu, k * U:(k + 1) * U]

            def emit_qk(s):
                qk = qw_pool.tile([128, 8 * U], sdt, tag="qw", name=f"qk_{s}")
                nc.sync.dma_start(out=qk[:], in_=xqnt_d[s])
                xqe = xqe_pool.tile([128, 2 * D], bf16, tag="xqe", name=f"xqe_{s}")
                nc.sync.dma_start(out=xqe[:], in_=xq_d[s])
                return qk, xqe

            order = list(reversed(range(NSLOT)))
            # first slot's queries lead the sync queue: the PE start gates
            # on them (qk-self sim group needs no resident keys)
            qk_next = emit_qk(order[0])
            cm_sb = res_pool.tile([128, 2 * 2 * U], f32, tag="cm")
            for h in range(2):
                nc.sync.dma_start(out=cm_sb[:, h * 2 * U:(h + 1) * 2 * U], in_=cm_d[h])
            ident = res_pool.tile([128, 128], bf16, tag="ident")
            make_identity(nc, ident[:])
            issue_xnt(6)
            if not trivial_affine:
                gb_sb = res_pool.tile([128, 2 * D], f32, tag="gb")
                g1 = res_pool.tile([1, 2 * D], f32, tag="g1")
                nc.sync.dma_start(out=g1[:, 0:D], in_=gain_d[:])
                nc.sync.dma_start(out=g1[:, D:2 * D], in_=bias_d[:])
                nc.vector.partition_broadcast(gb_sb[:], g1[:])

            for si, s in enumerate(order):
                W = WS[s]
                NJ = W // 2          # 512-wide sim tiles
                # split final tile only where needed: W=16 (unit 15 is not
                # resident) and W=2 (qk-self start needs no resident keys)
                split_last = W in (2, 16)
                sim_t = [None, None]
                tauc = [None, None]

                qk, xqe = qk_next
                if si + 1 < len(order):
                    qk_next = emit_qk(order[si + 1])
                issue_xnt(W + 4)

                # ---- phase 1: sim + max8 threshold, per stripe ----
                for h in range(2):
                    sim_t[h] = sim_pool.tile([128, 16 * U], f32, tag="sim", name=f"sim_{s}_{h}")
                    m8all = m8_pool.tile([128, 8 * 8], f32, tag="m8all")
                    for jg in range(NJ):
                        psim = psim_pool.tile([128, 512], f32, tag="psim", name=f"psim_{s}_{h}_{jg}")
                        if jg < NJ - 1 or not split_last:
                            for k in range(8):
                                nc.tensor.matmul(
                                    psim[:], qk[:, k * U + h * 128: k * U + h * 128 + 128],
                                    key_rhs(k, 2 * jg, 512),
                                    start=(k == 0), stop=(k == 7))
                        else:
                            # split final tile: two sequential accumulation
                            # groups in the same bank (interleaving is illegal).
                            # qk-self group first: no xnt dependency, so the
                            # very first slot can start before keys land.
                            for k in range(8):
                                nc.tensor.matmul(
                                    psim[:, U:2 * U], qk[:, k * U + h * 128: k * U + h * 128 + 128],
                                    qk[:, k * U:(k + 1) * U],
                                    start=(k == 0), stop=(k == 7))
                            for k in range(8):
                                nc.tensor.matmul(
                                    psim[:, 0:U], qk[:, k * U + h * 128: k * U + h * 128 + 128],
                                    key_rhs(k, W - 2, U),
                                    start=(k == 0), stop=(k == 7))
                        dst = sim_t[h][:, jg * 512:(jg + 1) * 512]
                        if jg < NJ - 1:
                            nc.scalar.copy(dst, psim[:])
                        else:
                            nc.vector.tensor_add(dst, psim[:], cm_sb[:, h * 2 * U:(h + 1) * 2 * U])
                        nc.vector.max(out=m8all[:, jg * 8:(jg + 1) * 8], in_=dst)
                    m8f = m8_pool.tile([128, 8], f32, tag="m8f")
                    nc.vector.max(out=m8f[:], in_=m8all[:, 0:NJ * 8])
                    tauc[h] = m8_pool.tile([128, 1], f32, tag="tauc", name=f"tauc_{s}_{h}")
                    nc.vector.tensor_scalar_max(tauc[h][:], m8f[:, 7:8], -2.0)

                # ---- phase 2: A-build + transpose + msg matmul ----
                # PE order T(h0,jg) M(h0,jg) T(h1,jg) M(h1,jg): the first
                # msg group covers tau(h1)'s DVE latency, so the PE never
                # waits at the stripe boundary.
                pmsg = [[pmsg_pool.tile([128, 512], f32, tag="pmsg", name=f"pmsg_{s}_{h}_{dh}")
                         for dh in range(2)] for h in range(2)]
                for jg in range(NJ):
                    xkt = []
                    for g in range(2):
                        jj8 = jg * 2 + g
                        if use_fp8:
                            xt = xk_pool.tile([128, 2, D], fp8, tag="xk")
                            nc.sync.dma_start(out=xt[:], in_=xk_d[jj8])
                        else:
                            xt = [None, None]
                            for i in range(2):
                                xt[i] = xk_pool.tile([128, D], bf16, tag="xk")
                                nc.sync.dma_start(
                                    out=xt[i][:],
                                    in_=xk_d[(jj8 * 2 + i) * 128:(jj8 * 2 + i + 1) * 128, :])
                        xkt.append(xt)
                    for h in range(2):
                        a_t = at_pool.tile([128, 512], bf16, tag="at")
                        nc.vector.tensor_scalar(
                            a_t[:], sim_t[h][:, jg * 512:(jg + 1) * 512],
                            tauc[h][:], float(msgc), op0=ALU.is_ge, op1=ALU.mult)
                        if use_fp8:
                            # transpose in bf16 (fp8 PE transpose needs
                            # stride-2 outputs); ACT copy converts to fp8
                            ptr = ptr_pool.tile([128, 4, 128], bf16, tag="ptr")
                            for t in range(4):
                                nc.tensor.transpose(ptr[:, t:t + 1, :],
                                                    a_t[:, t * 128:(t + 1) * 128], ident[:])
                            atr = atr_pool.tile([128, 4, 128], fp8, tag="atr",
                                                name=f"atr_{s}_{jg}_{h}")
                            nc.scalar.copy(atr[:], ptr[:])
                            for g in range(2):
                                for dh in range(2):
                                    nc.tensor.matmul(
                                        pmsg[h][dh][:], atr[:, 2 * g:2 * g + 2, :],
                                        xkt[g][:, :, dh * 512:(dh + 1) * 512],
                                        perf_mode=PM.DoubleRow,
                                        start=(jg == 0 and g == 0),
                                        stop=(jg == NJ - 1 and g == 1))
                        else:
                            ptr = ptr_pool.tile([128, 512], bf16, tag="ptr")
                            for t in range(4):
                                nc.tensor.transpose(ptr[:, t * 128:(t + 1) * 128],
                                                    a_t[:, t * 128:(t + 1) * 128], ident[:])
                            atr = atr_pool.tile([128, 512], bf16, tag="atr",
                                                name=f"atr_{s}_{jg}_{h}")
                            nc.scalar.copy(atr[:], ptr[:])
                            for g in range(2):
                                for i in range(2):
                                    jj = jg * 4 + g * 2 + i
                                    for dh in range(2):
                                        nc.tensor.matmul(
                                            pmsg[h][dh][:],
                                            atr[:, (g * 2 + i) * 128:(g * 2 + i + 1) * 128],
                                            xkt[g][i][:, dh * 512:(dh + 1) * 512],
                                            start=(jj == 0), stop=(jj == 4 * NJ - 1))

                # ---- phase 3: fused epilogue ----
                for h in range(2):
                    for dh in range(2):
                        pm = pmsg[h][dh]
                        nc.vector.scalar_tensor_tensor(
                            pm[:], xqe[:, h * D + dh * 512: h * D + (dh + 1) * 512],
                            float(mix), pm[:], op0=ALU.mult, op1=ALU.add)
                        if not trivial_affine:
                            nc.vector.tensor_mul(pm[:], pm[:], gb_sb[:, dh * 512:(dh + 1) * 512])
                            nc.vector.tensor_add(pm[:], pm[:], gb_sb[:, D + dh * 512: D + (dh + 1) * 512])
                        ot = ot_pool.tile([128, 512], bf16, tag="ot")
                        nc.scalar.activation(ot[:], pm[:],
                                             AF.Identity if _NOGELU else AF.Gelu)
                        nc.vector.tensor_scalar_mul(ot[:], ot[:], float(scale))
                        nc.sync.dma_start(
                            out=out_d[s * U + h * 128: s * U + (h + 1) * 128,
                                      dh * 512:(dh + 1) * 512],
                            in_=ot[:])
    nc.compile()
    return nc


def _gelu_exact(z):
    from scipy.special import erf
    z64 = z.astype(np.float64)
    return (0.5 * z64 * (1.0 + erf(z64 / np.sqrt(2.0)))).astype(np.float32)


def _pack_unit(rows):
    # rows: (256, 1024) f32 -> [128, 8*256]: out[p, k*U+c] = rows[c, k*128+p]
    return np.ascontiguousarray(
        rows.T.reshape(8, 128, U).transpose(1, 0, 2).reshape(128, 8 * U))


def kernel(x, gain, bias, log_mix, log_scale):
    x = np.asarray(x, dtype=np.float32)
    gain = np.asarray(gain, dtype=np.float32)
    bias = np.asarray(bias, dtype=np.float32)
    mix = float(1.0 / (1.0 + np.exp(-np.float64(log_mix))))
    scale = float(np.log1p(np.exp(np.float64(log_scale))) + 0.01)
    trivial = bool(np.all(gain == 1.0) and np.all(bias == 0.0))

    key = (SIM_DT, MSG_DT, trivial, round(mix, 12), round(scale, 12))
    if key not in _PROG_CACHE:
        _PROG_CACHE[key] = _build_program(SIM_DT, MSG_DT, trivial, mix, scale)
    nc = _PROG_CACHE[key]
    use_fp8 = MSG_DT == "fp8"

    norms = np.sqrt((x.astype(np.float32) ** 2).sum(-1, keepdims=True)).astype(np.float32)
    xn = x / (norms + np.float32(1e-8))

    in_maps = []
    qunits = []  # per core: list of q unit index per slot
    for c in range(8):
        b, p = c // 2, c % 2
        kus = [WS[s] - 1 - p for s in range(NSLOT)]
        qunits.append(kus)
        xb, xnb = x[b], xn[b]
        sim_np_dt = np.float16 if SIM_DT == "fp16" else np.float32
        xnt_np = np.stack([_pack_unit(xnb[u * U:(u + 1) * U]) for u in range(NRES)]).astype(sim_np_dt)
        xqnt_np = np.stack([_pack_unit(xnb[ku * U:(ku + 1) * U]) for ku in kus]).astype(sim_np_dt)
        xq_np = np.stack([
            np.concatenate([xb[ku * U:ku * U + 128], xb[ku * U + 128:(ku + 1) * U]], axis=1)
            for ku in kus]).astype(ml_dtypes.bfloat16)
        if use_fp8:
            xk_np = np.stack([
                np.concatenate([xb[g * 256:g * 256 + 128], xb[g * 256 + 128:(g + 1) * 256]], axis=1)
                for g in range(T // 256)]).astype(ml_dtypes.float8_e4m3)
        else:
            xk_np = xb.astype(ml_dtypes.bfloat16)
        r = np.arange(128)[:, None]
        f = np.arange(2 * U)[None, :]
        cm = np.zeros((2, 128, 2 * U), dtype=np.float32)
        for h in range(2):
            row = h * 128 + r
            allowed = f <= (row + U) if p == 0 else f <= row
            cm[h] = np.where(allowed, 0.0, NEG)
        m = {"xnt": xnt_np, "xqnt": xqnt_np, "xk": xk_np, "xq": xq_np, "cmask": cm}
        if not trivial:
            m["gain"] = gain.reshape(1, D)
            m["bias"] = bias.reshape(1, D)
        in_maps.append(m)

    global _LAST_IN_MAPS
    _LAST_IN_MAPS = in_maps
    res = run_bass_kernel_spmd(nc, in_maps, list(range(8)), trace=False)

    y = np.empty((B, T, D), dtype=np.float32)
    for c in range(8):
        b = c // 2
        oc = np.asarray(res.results[c]["out"]).astype(np.float32)
        for s, ku in enumerate(qunits[c]):
            y[b, ku * U:(ku + 1) * U] = oc[s * U:(s + 1) * U]

    # exact host fixup for rows with fewer than K neighbors (q < 7)
    for b in range(B):
        nq = K - 1
        msg = np.cumsum(x[b, :nq], axis=0) / np.arange(1, nq + 1, dtype=np.float32)[:, None]
        blended = np.float32(mix) * x[b, :nq] + np.float32(1.0 - mix) * msg
        y[b, :nq] = _gelu_exact(blended * gain + bias) * np.float32(scale)

    return y
